# revision 17
# baseline (speedup 1.0000x reference)
"""Haar DWT (single-level) Bass kernel for Trainium2, 8-core data-parallel.

Input  x: [8, 64, 512, 512] f32
Output (ll, lh, hl, hh): each [8, 64, 256, 256] f32

Math (per 2x2 block a=x[2i,2j], b=x[2i,2j+1], c=x[2i+1,2j], d=x[2i+1,2j+1]):
    ll = 0.5(a+b+c+d), lh = 0.5(a-b+c-d), hl = 0.5(a+b-c-d), hh = 0.5(a-b-c+d)

Sharding: pure data-parallel over batch; core k processes x[k] ([64,512,512]).

Per-core layout: each iteration handles 2 channels. SBUF tile xt[128, 4096]
holds 2 images; partition p, free = (img, chunk, rowpar, w) where DRAM row
h = chunk*256 + 2p + rowpar. So the column (H) butterfly is a free-dim offset
(rowpar 0 vs 1) and the row (W) butterfly is a stride-2 free-dim access.

Pipeline per iteration:
  sync  : DMA load xt (2MB, contiguous 4KB runs per partition)
  scalar: bs = 0.5 * odd rows (ACT)
  vector: st = (even*0.5) + bs ; dt = (even*0.5) - bs   (scalar_tensor_tensor)
          ll = st_e + st_o ; lh = st_e - st_o ; hl = dt_e + dt_o ; hh = dt_e - dt_o
  gpsimd: 4 DMA stores (separate queue so store-waits don't stall loads)
"""

import numpy as np

import concourse.bass as bass
import concourse.bacc as bacc
import concourse.mybir as mybir
import concourse.tile as tile
from concourse.bass_utils import run_bass_kernel_spmd

B, C, H, W = 8, 64, 512, 512
H2, W2 = H // 2, W // 2
N_CORES = 8
IPI = 2  # images (channels) per iteration
F32 = mybir.dt.float32
OUT_NAMES = ("ll", "lh", "hl", "hh")

_cached_nc = None


def _build(reps: int = 1):
    """reps>1 repeats the whole pass back-to-back inside one NEFF (timing)."""
    nc = bacc.Bacc()
    x = nc.dram_tensor("x", [C, H, W], F32, kind="ExternalInput")
    outs = {
        nm: nc.dram_tensor(nm, [C, H2, W2], F32, kind="ExternalOutput")
        for nm in OUT_NAMES
    }

    add = mybir.AluOpType.add
    sub = mybir.AluOpType.subtract
    mult = mybir.AluOpType.mult

    with tile.TileContext(nc) as tc:
        with (
            tc.tile_pool(name="xp", bufs=3) as xp,
            tc.tile_pool(name="bsp", bufs=2) as bsp,
            tc.tile_pool(name="sdp", bufs=3) as sdp,
            tc.tile_pool(name="op", bufs=3) as op,
        ):
            for it in range(reps * (C // IPI)):
                c0 = (it % (C // IPI)) * IPI
                # ---- load 2 images: [128, 4096]
                xt = xp.tile([128, IPI * 2048], F32)
                # h = 4p + 2c + r: each partition's load is one contiguous
                # 8KB run per image; each store run is contiguous 2KB.
                src = x[c0 : c0 + IPI].rearrange(
                    "i (p c r) w -> p i c r w", p=128, c=2, r=2
                )
                dst_x = xt[:].rearrange("p (i c r w) -> p i c r w", i=IPI, c=2, r=2, w=W)
                nc.sync.dma_start(out=dst_x, in_=src)

                # ---- ACT: xs = 0.5 * x (one dense op; keeps DVE ops plain TT,
                # since the STT ISA format can't encode 2 semaphore waits)
                xs = bsp.tile([128, IPI * 2048], F32)
                nc.scalar.mul(xs[:], xt[:], 0.5)

                xv = xs[:].rearrange("p (i c r w) -> p i c r w", i=IPI, c=2, r=2, w=W)
                ev = xv[:, :, :, 0]  # even rows  [128, IPI, 2, 512]
                ov = xv[:, :, :, 1]  # odd rows

                # ---- DVE stage 1 (column butterfly)
                st = sdp.tile([128, IPI * 1024], F32, tag="st")
                dt = sdp.tile([128, IPI * 1024], F32, tag="dt")
                stv = st[:].rearrange("p (i c w) -> p i c w", i=IPI, c=2, w=W)
                dtv = dt[:].rearrange("p (i c w) -> p i c w", i=IPI, c=2, w=W)
                nc.vector.tensor_tensor(stv, ev, ov, add)
                nc.vector.tensor_tensor(dtv, ev, ov, sub)

                # ---- DVE stage 2 (row butterfly, stride-2)
                sv = st[:].rearrange("p (i c j t) -> p i c j t", i=IPI, c=2, j=W2, t=2)
                dv = dt[:].rearrange("p (i c j t) -> p i c j t", i=IPI, c=2, j=W2, t=2)
                se, so = sv[:, :, :, :, 0], sv[:, :, :, :, 1]
                de, do = dv[:, :, :, :, 0], dv[:, :, :, :, 1]
                for nm, e, o, alu, dma_eng in (
                    ("ll", se, so, add, nc.scalar),
                    ("lh", se, so, sub, nc.gpsimd),
                    ("hl", de, do, add, nc.scalar),
                    ("hh", de, do, sub, nc.gpsimd),
                ):
                    t = op.tile([128, IPI * 512], F32, tag=nm)
                    tv = t[:].rearrange("p (i c j) -> p i c j", i=IPI, c=2, j=W2)
                    nc.vector.tensor_tensor(tv, e, o, alu)
                    # stores split across scalar-HWDGE and gpsimd-SWDGE rings
                    # (never the sync ring, so store-waits can't block loads)
                    dst = outs[nm][c0 : c0 + IPI].rearrange(
                        "i (p c) j -> p i c j", p=128, c=2
                    )
                    dma_eng.dma_start(out=dst, in_=tv)
    nc.finalize()  # Bacc: runs compile() — reg alloc + event-semaphore wait split
    return nc


def _get_nc():
    global _cached_nc
    if _cached_nc is None:
        _cached_nc = _build()
    return _cached_nc


def kernel(x: np.ndarray):
    x = np.asarray(x)
    assert x.shape == (B, C, H, W) and x.dtype == np.float32, (x.shape, x.dtype)
    x = np.ascontiguousarray(x)
    nc = _get_nc()
    in_maps = [{"x": x[k]} for k in range(N_CORES)]
    res = run_bass_kernel_spmd(nc, in_maps, core_ids=list(range(N_CORES))).results
    return tuple(
        np.stack([res[k][nm] for k in range(N_CORES)], axis=0) for nm in OUT_NAMES
    )


# revision 23
# speedup vs baseline: 1.0155x; 1.0155x over previous
"""Haar DWT (single-level) Bass kernel for Trainium2, 8-core data-parallel.

Input  x: [8, 64, 512, 512] f32
Output (ll, lh, hl, hh): each [8, 64, 256, 256] f32

Math (per 2x2 block a=x[2i,2j], b=x[2i,2j+1], c=x[2i+1,2j], d=x[2i+1,2j+1]):
    ll = 0.5(a+b+c+d), lh = 0.5(a-b+c-d), hl = 0.5(a+b-c-d), hh = 0.5(a-b-c+d)

Sharding: pure data-parallel over batch; core k processes x[k] ([64,512,512]).

Per-core layout: each iteration handles 2 channels. SBUF tile xt[128, 4096]
holds 2 images; partition p, free = (img, chunk, rowpar, w) where DRAM row
h = chunk*256 + 2p + rowpar. So the column (H) butterfly is a free-dim offset
(rowpar 0 vs 1) and the row (W) butterfly is a stride-2 free-dim access.

Pipeline per iteration:
  sync  : DMA load xt (2MB, contiguous 4KB runs per partition)
  scalar: bs = 0.5 * odd rows (ACT)
  vector: st = (even*0.5) + bs ; dt = (even*0.5) - bs   (scalar_tensor_tensor)
          ll = st_e + st_o ; lh = st_e - st_o ; hl = dt_e + dt_o ; hh = dt_e - dt_o
  gpsimd: 4 DMA stores (separate queue so store-waits don't stall loads)
"""

import numpy as np

import concourse.bass as bass
import concourse.bacc as bacc
import concourse.mybir as mybir
import concourse.tile as tile
from concourse.bass_utils import run_bass_kernel_spmd

B, C, H, W = 8, 64, 512, 512
H2, W2 = H // 2, W // 2
N_CORES = 8
IPI = 2  # images (channels) per iteration
F32 = mybir.dt.float32
OUT_NAMES = ("ll", "lh", "hl", "hh")

_cached_nc = None


def _build(reps: int = 1):
    """reps>1 repeats the whole pass back-to-back inside one NEFF (timing)."""
    nc = bacc.Bacc()
    x = nc.dram_tensor("x", [C, H, W], F32, kind="ExternalInput")
    outs = {
        nm: nc.dram_tensor(nm, [C, H2, W2], F32, kind="ExternalOutput")
        for nm in OUT_NAMES
    }

    add = mybir.AluOpType.add
    sub = mybir.AluOpType.subtract
    mult = mybir.AluOpType.mult

    with tile.TileContext(nc) as tc:
        with (
            tc.tile_pool(name="xp", bufs=3) as xp,
            tc.tile_pool(name="bsp", bufs=2) as bsp,
            tc.tile_pool(name="sdp", bufs=2) as sdp,
            tc.tile_pool(name="op", bufs=2) as op,
        ):
            ot = {}  # per-subband output tiles, batched over 2 iterations
            for it in range(reps * (C // IPI)):
                c0 = (it % (C // IPI)) * IPI
                half = it % 2  # which half of the batched output tile
                # ---- load 2 images: [128, 4096]
                xt = xp.tile([128, IPI * 2048], F32)
                # h = 4p + 2c + r: each partition's load is one contiguous
                # 8KB run per image; each store run is contiguous 2KB.
                src = x[c0 : c0 + IPI].rearrange(
                    "i (p c r) w -> p i c r w", p=128, c=2, r=2
                )
                dst_x = xt[:].rearrange("p (i c r w) -> p i c r w", i=IPI, c=2, r=2, w=W)
                nc.sync.dma_start(out=dst_x, in_=src)

                # ---- ACT: xs = 0.5 * x (one dense op; keeps DVE ops plain TT,
                # since the STT ISA format can't encode 2 semaphore waits)
                xs = bsp.tile([128, IPI * 2048], F32)
                nc.scalar.mul(xs[:], xt[:], 0.5)

                xv = xs[:].rearrange("p (i c r w) -> p i c r w", i=IPI, c=2, r=2, w=W)
                ev = xv[:, :, :, 0]  # even rows  [128, IPI, 2, 512]
                ov = xv[:, :, :, 1]  # odd rows

                # ---- DVE stage 1 (column butterfly)
                st = sdp.tile([128, IPI * 1024], F32, tag="st")
                dt = sdp.tile([128, IPI * 1024], F32, tag="dt")
                stv = st[:].rearrange("p (i c w) -> p i c w", i=IPI, c=2, w=W)
                dtv = dt[:].rearrange("p (i c w) -> p i c w", i=IPI, c=2, w=W)
                nc.vector.tensor_tensor(stv, ev, ov, add)
                nc.vector.tensor_tensor(dtv, ev, ov, sub)

                # ---- DVE stage 2 (row butterfly, stride-2)
                sv = st[:].rearrange("p (i c j t) -> p i c j t", i=IPI, c=2, j=W2, t=2)
                dv = dt[:].rearrange("p (i c j t) -> p i c j t", i=IPI, c=2, j=W2, t=2)
                se, so = sv[:, :, :, :, 0], sv[:, :, :, :, 1]
                de, do = dv[:, :, :, :, 0], dv[:, :, :, :, 1]
                for nm, e, o, alu in (
                    ("ll", se, so, add),
                    ("lh", se, so, sub),
                    ("hl", de, do, add),
                    ("hh", de, do, sub),
                ):
                    if half == 0:
                        ot[nm] = op.tile(
                            [128, 2 * IPI * 512], F32, tag=nm, name=f"ot_{nm}"
                        )
                    t = ot[nm]
                    tv = t[:, half * IPI * 512 : (half + 1) * IPI * 512].rearrange(
                        "p (i c j) -> p i c j", i=IPI, c=2, j=W2
                    )
                    nc.vector.tensor_tensor(tv, e, o, alu)
                    if half == 1:
                        # 1MB store of 4 channels on the scalar HWDGE ring:
                        # bigger write bursts; keeps store-waits off the sync
                        # ring so they never block load prefetch
                        dst = outs[nm][c0 - IPI : c0 + IPI].rearrange(
                            "i (p c) j -> p i c j", p=128, c=2
                        )
                        src = t[:].rearrange(
                            "p (i c j) -> p i c j", i=2 * IPI, c=2, j=W2
                        )
                        nc.scalar.dma_start(out=dst, in_=src)
    nc.finalize()  # Bacc: runs compile() — reg alloc + event-semaphore wait split
    return nc


def _get_nc():
    global _cached_nc
    if _cached_nc is None:
        _cached_nc = _build()
    return _cached_nc


def kernel(x: np.ndarray):
    x = np.asarray(x)
    assert x.shape == (B, C, H, W) and x.dtype == np.float32, (x.shape, x.dtype)
    x = np.ascontiguousarray(x)
    nc = _get_nc()
    in_maps = [{"x": x[k]} for k in range(N_CORES)]
    res = run_bass_kernel_spmd(nc, in_maps, core_ids=list(range(N_CORES))).results
    return tuple(
        np.stack([res[k][nm] for k in range(N_CORES)], axis=0) for nm in OUT_NAMES
    )
